# revision 8
# baseline (speedup 1.0000x reference)
"""Trainium2 Bass kernel for DeiT self-attention with channel-pruning masks.

Reference computation (B=16, S=577, HID=768, H=12, D=64, N_KEEP=576):
    q/k/v = hs @ W + b            [B,S,576]
    scatter channels to [B,S,768] at {q,k,v}_idx, split into 12 heads of 64
    softmax attention per (b, h), concat heads, gather v_idx channels.

Strategy:
  - The channel scatters are folded into the weight matrices on the host
    (zero columns at dropped channels), so the device kernel is a dense
    attention over the full 768-channel layout.
  - Data-parallel over batch: 8 cores x 2 images each.
  - Per core the device computes (token count T = 2*577 = 1154):
      V_aug    = hsT-stationary matmul            [T, 780]       (SBUF)
                 (per head: 64 value cols + one bias-column of ones)
      then per output-channel chunk i (128 channels = 2 heads):
        Q_i^T, K_i^T = W-stationary matmuls       [128, T]
        attention for heads 2i, 2i+1, both images:
          S^T = K_h^T x Q_h            [ktok, qtok] PSUM (f32r matmul)
          E = exp(S^T / 8)             ScalarE, PSUM -> SBUF (f32r)
          ctxU^T|Z = [V_h|1]^T x E     accumulated over ktok chunks
        so ScalarE exp overlaps the next chunk's projection matmuls.
  - Device output is unnormalized: outA[65h:65h+64] = ctxU^T rows, and
    outA[65h+64] = Z (softmax denominators). The host divides, transposes,
    and gathers v_idx.
  - Matmuls run as float32r (fp32 with 11-bit stored mantissa, full PE
    streaming rate; inputs are pre-rounded host-side).
"""

import numpy as np

B, S, HID = 16, 577, 768
H, D = 12, 64
N_KEEP = 576
NCORES = 8
BPC = B // NCORES          # images per core
TOK = BPC * S              # tokens per core
VW = H * (D + 1)           # 780: V columns augmented with per-head ones column
P = 128
ICH = HID // P             # 6 input-channel chunks
OCH = HID // P             # 6 q/k output-channel chunks
TOK_TILES = [(0, 386), (386, 386), (772, 382)]      # projection moving tiles (even)
KCHUNKS = [(0, 128), (128, 128), (256, 128), (384, 128), (512, 65)]  # per image
# (q_offset, scores width (even), ctx width) per image; qt1 is shifted +289
QTILES = [(0, 290, 290), (289, 290, 288)]
TOK_P = 1160                                        # q/k token dim padded for qt1 reads

_NC_CACHE = {}


def _build_nc(use_f32r=True):
    import concourse.bacc as bacc
    import concourse.mybir as mybir
    import concourse.tile as tile

    f32 = mybir.dt.float32
    mm_dt = mybir.dt.float32r if use_f32r else mybir.dt.float32

    nc = bacc.Bacc("TRN2", target_bir_lowering=False)

    hsT = nc.dram_tensor("hsT", [HID, TOK], mm_dt, kind="ExternalInput")
    wq = nc.dram_tensor("wq", [HID, HID], mm_dt, kind="ExternalInput")
    wk = nc.dram_tensor("wk", [HID, HID], mm_dt, kind="ExternalInput")
    wv = nc.dram_tensor("wv", [HID, VW], mm_dt, kind="ExternalInput")
    bq = nc.dram_tensor("bq", [HID], f32, kind="ExternalInput")
    bk = nc.dram_tensor("bk", [HID], f32, kind="ExternalInput")
    bvb = nc.dram_tensor("bvb", [P, VW], f32, kind="ExternalInput")
    outA = nc.dram_tensor("outA", [VW, TOK], f32, kind="ExternalOutput")

    def mm(out_ps, lhsT, rhs, start, stop):
        nc.tensor.matmul(out_ps, lhsT, rhs, start=start, stop=stop)

    with tile.TileContext(nc) as tc:
        Exp = mybir.ActivationFunctionType.Exp
        with (
            tc.tile_pool(name="big", bufs=1) as big,
            tc.tile_pool(name="psa", bufs=2, space="PSUM") as psa,   # proj accums + ctx chains
            tc.tile_pool(name="psb", bufs=3, space="PSUM") as psb,   # score pairs (2 banks each)
            tc.tile_pool(name="wpool", bufs=2) as wpool,
            tc.tile_pool(name="epool", bufs=4) as epool,
            tc.tile_pool(name="opool", bufs=4) as opool,
        ):
            # ---- persistent SBUF tensors ----
            hsT_sb = big.tile([P, ICH, TOK], mm_dt)
            hsT_r = hsT.rearrange("(c p) t -> p c t", p=P)
            for b0 in range(BPC):
                for j0, (koff0, kcs0) in enumerate(KCHUNKS):
                    t0_ = b0 * S + koff0
                    nc.sync.dma_start(
                        hsT_sb[:, :, t0_ : t0_ + kcs0], hsT_r[:, :, t0_ : t0_ + kcs0]
                    )
            bvb_sb = big.tile([P, VW], f32)
            nc.sync.dma_start(bvb_sb[:], bvb[:])
            bq_sb = big.tile([P, OCH], f32)
            nc.sync.dma_start(bq_sb[:], bq.rearrange("(c p) -> p c", p=P))
            bk_sb = big.tile([P, OCH], f32)
            nc.sync.dma_start(bk_sb[:], bk.rearrange("(c p) -> p c", p=P))

            q_sb = big.tile([P, OCH, TOK_P], mm_dt)
            k_sb = big.tile([P, OCH, TOK], mm_dt)
            v_sb = big.tile([P, BPC * len(KCHUNKS), VW], mm_dt)
            # zero the padded token tail once (read by qt1 score matmuls for b=1)
            nc.vector.memset(q_sb[:, :, TOK:].bitcast(f32), 0.0)

            # ---- V projection (wv lives only here) ----
            with tc.tile_pool(name="pwv", bufs=1) as pwv:
                wv_sb = pwv.tile([P, ICH, VW], mm_dt)
                nc.sync.dma_start(wv_sb[:], wv.rearrange("(c p) n -> p c n", p=P))
                VT = VW // 2  # 390, head-aligned (6 heads x 65)
                for b in range(BPC):
                    for j, (koff, kcs) in enumerate(KCHUNKS):
                        toff = b * S + koff
                        for n in range(2):
                            vp = psa.tile([P, 512], f32, tag="ps", name="vp")[:kcs, :VT]
                            for k in range(ICH):
                                mm(
                                    vp,
                                    hsT_sb[:, k, toff : toff + kcs],
                                    wv_sb[:, k, n * VT : (n + 1) * VT],
                                    start=(k == 0),
                                    stop=(k == ICH - 1),
                                )
                            nc.vector.tensor_add(
                                out=v_sb[:kcs, b * 5 + j, n * VT : (n + 1) * VT],
                                in0=vp,
                                in1=bvb_sb[:kcs, n * VT : (n + 1) * VT],
                            )

            # ---- interleaved Q/K projection + attention ----
            # per chunk i: project Q_i/K_i, then 4 units (2 heads x 2 images).
            # Emission is software-pipelined: score+exp blocks for all units of
            # chunk i, then ctx blocks interleaved with chunk i+1 projections.
            def load_w(i, w_dram):
                w_sb = wpool.tile([P, ICH, P], mm_dt, tag="w", name="w_sb")
                nc.sync.dma_start(
                    w_sb[:], w_dram.rearrange("(c p) n -> p c n", p=P)[:, :, i * P : (i + 1) * P]
                )
                return w_sb

            def emit_proj(i, w_sb, b_sb, dst):
                for toff, tcs in TOK_TILES:
                    qp = psa.tile([P, 512], f32, tag="ps", name="qp")[:, :tcs]
                    for k in range(ICH):
                        mm(
                            qp,
                            w_sb[:, k, :],
                            hsT_sb[:, k, toff : toff + tcs],
                            start=(k == 0),
                            stop=(k == ICH - 1),
                        )
                    nc.vector.tensor_add(
                        out=dst[:, i, toff : toff + tcs],
                        in0=qp,
                        in1=b_sb[:, i : i + 1].to_broadcast((P, tcs)),
                    )

            NK = len(KCHUNKS)

            def emit_sp_block(i, h, b):
                e_sb = epool.tile([P, NK, 2, 290], mm_dt, tag="e")
                pb = 64 * (h % 2)
                for c, (ko, kcs) in enumerate(KCHUNKS):
                    sp2 = psb.tile([P, 1024], f32, tag="sp", name="sp2")
                    for qt, (qo, sw, cw) in enumerate(QTILES):
                        mm(
                            sp2[:kcs, qt * 512 : qt * 512 + sw],
                            k_sb[pb : pb + 64, i, b * S + ko : b * S + ko + kcs],
                            q_sb[pb : pb + 64, i, b * S + qo : b * S + qo + sw],
                            start=True,
                            stop=True,
                        )
                    nc.scalar.activation(
                        e_sb[:kcs, c, :, :],
                        sp2.rearrange("p (two q) -> p two q", two=2)[:kcs, :, :290],
                        Exp,
                        scale=0.125,
                    )
                return e_sb

            def emit_cp_block(i, h, b, e_sb):
                for qt, (qo, sw, cw) in enumerate(QTILES):
                    cp = psa.tile([P, 512], f32, tag="ps", name="cp")[:65, :cw]
                    for c, (ko, kcs) in enumerate(KCHUNKS):
                        mm(
                            cp,
                            v_sb[:kcs, b * 5 + c, h * 65 : (h + 1) * 65],
                            e_sb[:kcs, c, qt, :cw],
                            start=(c == 0),
                            stop=(c == NK - 1),
                        )
                    o_sb = opool.tile([65, 512], f32, tag="o", name="o_sb")[:, :cw]
                    nc.vector.tensor_copy(o_sb, cp)
                    ow = cw if qt == 1 else 289
                    nc.sync.dma_start(
                        outA[h * 65 : (h + 1) * 65, b * S + qo : b * S + qo + ow],
                        o_sb[:, :ow],
                    )

            wq0 = load_w(0, wq)
            wk0 = load_w(0, wk)
            emit_proj(0, wq0, bq_sb, q_sb)
            emit_proj(0, wk0, bk_sb, k_sb)
            wq_next = load_w(1, wq)
            wk_next = load_w(1, wk)
            for i in range(OCH):
                units = [(2 * i, 0), (2 * i, 1), (2 * i + 1, 0), (2 * i + 1, 1)]
                es = [emit_sp_block(i, h, b) for h, b in units]
                emit_cp_block(i, *units[0], es[0])
                emit_cp_block(i, *units[1], es[1])
                if i + 1 < OCH:
                    emit_proj(i + 1, wq_next, bq_sb, q_sb)
                emit_cp_block(i, *units[2], es[2])
                if i + 1 < OCH:
                    emit_proj(i + 1, wk_next, bk_sb, k_sb)
                    if i + 2 < OCH:
                        wq_next = load_w(i + 2, wq)
                        wk_next = load_w(i + 2, wk)
                emit_cp_block(i, *units[3], es[3])

    nc.compile()
    return nc


def _get_nc(use_f32r=True):
    key = ("nc", use_f32r)
    if key not in _NC_CACHE:
        _NC_CACHE[key] = _build_nc(use_f32r)
    return _NC_CACHE[key]


def _round_fp32r(x):
    """Round fp32 -> fp32r bit pattern (11-bit stored mantissa, RNE), fp32 container."""
    u = np.ascontiguousarray(x).view(np.uint32).astype(np.uint64)
    u = u + 0x7FF + ((u >> 12) & 1)
    return (u & ~np.uint64(0xFFF)).astype(np.uint32).view(np.float32).reshape(x.shape)


def _make_in_maps(hidden_states, Wq, bq, Wk, bk, Wv, bv, q_idx, k_idx, v_idx,
                  use_f32r=True):
    f32 = np.float32
    hs = np.asarray(hidden_states, f32)
    q_idx = np.asarray(q_idx).astype(np.int64)
    k_idx = np.asarray(k_idx).astype(np.int64)
    v_idx = np.asarray(v_idx).astype(np.int64)

    # fold channel scatters into full-width weights
    wq_full = np.zeros((HID, HID), f32)
    wq_full[:, q_idx] = np.asarray(Wq, f32)
    bq_full = np.zeros(HID, f32)
    bq_full[q_idx] = np.asarray(bq, f32)
    wk_full = np.zeros((HID, HID), f32)
    wk_full[:, k_idx] = np.asarray(Wk, f32)
    bk_full = np.zeros(HID, f32)
    bk_full[k_idx] = np.asarray(bk, f32)

    wv_full = np.zeros((HID, HID), f32)
    wv_full[:, v_idx] = np.asarray(Wv, f32)
    bv_full = np.zeros(HID, f32)
    bv_full[v_idx] = np.asarray(bv, f32)
    # augmented V layout: per head 64 value cols + a ones column (softmax denom)
    wv_aug = np.zeros((HID, VW), f32)
    bv_aug = np.zeros(VW, f32)
    for h in range(H):
        wv_aug[:, h * 65 : h * 65 + 64] = wv_full[:, h * 64 : (h + 1) * 64]
        bv_aug[h * 65 : h * 65 + 64] = bv_full[h * 64 : (h + 1) * 64]
        bv_aug[h * 65 + 64] = 1.0
    bvb = np.broadcast_to(bv_aug, (P, VW)).copy()

    if use_f32r:
        wq_full = _round_fp32r(wq_full)
        wk_full = _round_fp32r(wk_full)
        wv_aug = _round_fp32r(wv_aug)

    in_maps = []
    for c in range(NCORES):
        hsT = np.ascontiguousarray(
            hs[c * BPC : (c + 1) * BPC].reshape(TOK, HID).T
        )
        if use_f32r:
            hsT = _round_fp32r(hsT)
        in_maps.append(
            {
                "hsT": hsT,
                "wq": wq_full,
                "wk": wk_full,
                "wv": wv_aug,
                "bq": bq_full,
                "bk": bk_full,
                "bvb": bvb,
            }
        )
    return in_maps, v_idx


def _assemble_output(results, v_idx):
    ctx = np.empty((B, S, HID), np.float32)
    for c in range(NCORES):
        aug = results[c]["outA"].reshape(H, D + 1, TOK)
        ctxu = aug[:, :D, :] / aug[:, D:, :]            # [H, D, TOK]
        ctx[c * BPC : (c + 1) * BPC] = (
            ctxu.reshape(HID, TOK).T.reshape(BPC, S, HID)
        )
    return np.ascontiguousarray(ctx[:, :, v_idx])


def run(inputs, trace=False, use_f32r=True, **spmd_kwargs):
    """Full pipeline; returns (output, BassKernelResults)."""
    from concourse import bass_utils

    in_maps, v_idx = _make_in_maps(**inputs, use_f32r=use_f32r)
    nc = _get_nc(use_f32r)
    res = bass_utils.run_bass_kernel_spmd(
        nc, in_maps, core_ids=list(range(NCORES)), trace=trace, **spmd_kwargs
    )
    return _assemble_output(res.results, v_idx), res


def kernel(**inputs):
    out, _ = run(inputs, trace=False)
    return out
